# revision 31
# baseline (speedup 1.0000x reference)
"""Expert-parallel batched-expert FFN kernel for Trainium2 (8 NeuronCores).

Reference computation (per expert e):
    y = relu(x[e] @ fc1_w[e] + fc1_b[e]) @ fc2_w[e] + fc2_b[e]

Sharding: E=8 experts, one expert per core (expert parallel, no collectives).

Per-core algorithm (T=2048 tokens, D=1024, H=4096), token-group-outer:
  - All large operands are repacked host-side (with the fp32->fp16 cast)
    into the exact SBUF slab layouts the kernel consumes, so every device
    DMA is one ~1MB transfer with >=1KB-contiguous per-partition rows:
      xt  [c4*128+p][k][t]  (x transposed to [D,T], chunked by 512 tokens)
      w1  [b*128+p][k][h]   (FC1 lhsT tiles, per 512-wide h block)
      w2  [b*128+p][hk][d]  (FC2 rhs tiles, per block)
  - Both weight matrices are SBUF-resident (16MB fp16 = 128KB/partition);
    they stream in once behind the ramp-critical slabs (x c0 + w1(0)).
  - Outer loop over 4 groups of 512 tokens.  Per group:
      FC1: for each of 32 h-tiles, accumulate 8 k-tile matmuls in PSUM,
           relu (+b1, fused) drains to a [128, 512] yT tile (scalar eng).
      FC2: for each (ti, dc) output tile, accumulate ALL 32 h-tiles in a
           single PSUM pass, then one DVE add (+b2 broadcast) drains to
           SBUF and the 256KB store issues immediately.
    So there is no cross-block SBUF accumulator, x chunks 1-3 stay out of
    the DMA ramp, and output stores spread across the whole run instead
    of flushing 8MB at the end.
  - x chunks use a 2-slot SBUF window; the slot-reuse WAR dependency
    auto-delays chunk c+2's DMA until group c's FC1 finished.
  - Matmul operands are fp16 (m10): inputs round to ~2^-11 relative; all
    accumulation is fp32 in PSUM.  Measured end-to-end L2 relative error
    vs the fp32 reference is ~4e-4.
  - Dependency-free REAL matmuls (not transposes, which don't count as
    PE-busy for the HAM clock gate) at t=0 bring the PE clock to 8/8
    during the DMA-bound lead-in so FC1 starts at full rate.
"""

from contextlib import ExitStack

import numpy as np

import concourse.bass as bass
import concourse.bacc as bacc
import concourse.mybir as mybir
import concourse.tile as tile
from concourse.bass_utils import run_bass_kernel_spmd

E, T, D, H = 8, 2048, 1024, 4096
NCORES = 8
HB = 512           # h per weight block
FP = mybir.dt.float32
FP16 = mybir.dt.float16
RELU = mybir.ActivationFunctionType.Relu

N_BLK = H // HB                # 8   weight blocks
N_HI = HB // 128               # 4   h-tiles per block
N_HK = H // 128                # 32  h-tiles total
N_KI = D // 128                # 8   k-tiles for FC1
N_DC = D // 512                # 2   512-col chunks of D
N_C4 = T // 512                # 4   512-token groups
N_TG = 4                       # ti tiles per token group
N_JUNK = 52                    # HAM warm-up matmuls at t=0


def _emit_kernel(tc, out, xt, w1, b1, w2, b2):
    nc = tc.nc
    with ExitStack() as ctx:
        singles = ctx.enter_context(tc.tile_pool(name="singles", bufs=1))
        xt_pool = ctx.enter_context(tc.tile_pool(name="xt", bufs=1))
        yt_pool = ctx.enter_context(tc.tile_pool(name="yt", bufs=1))
        st_pool = ctx.enter_context(tc.tile_pool(name="st", bufs=4))
        w1_pool = ctx.enter_context(tc.tile_pool(name="w1", bufs=1))
        w2_pool = ctx.enter_context(tc.tile_pool(name="w2", bufs=1))
        psum = ctx.enter_context(tc.tile_pool(name="psum", bufs=4, space="PSUM"))

        # ---- ramp-critical DMA order: x c0 leads the sync ring, w1
        # blocks lead the scalar ring; w2 + the rest queue behind. ----
        # sync ring:   x c0 | w2(0..7) | b2b | x c1   (+ half the stores)
        # scalar ring: b1t | w1(0..7) | x c2 | x c3   (+ half the stores)
        xTc = [None] * N_C4

        def emit_xload(c4, eng):
            xTc[c4] = xt_pool.tile([128, N_KI, 512], FP16, tag=f"xt{c4 % 2}",
                                   name=f"xT{c4}")
            eng.dma_start(out=xTc[c4], in_=xt[c4 * 128:(c4 + 1) * 128, :, :])

        emit_xload(0, nc.sync)

        b1t = singles.tile([128, N_HK], FP)
        nc.scalar.dma_start(out=b1t, in_=b1)

        w1b = []
        for b in range(N_BLK):
            wb = w1_pool.tile([128, N_KI, HB], FP16, tag=f"w1b{b}",
                              name=f"w1b{b}")
            nc.scalar.dma_start(out=wb, in_=w1[b * 128:(b + 1) * 128, :, :])
            w1b.append(wb)

        w2b = []
        for b in range(N_BLK):
            wb = w2_pool.tile([128, N_HI, D], FP16, tag=f"w2b{b}",
                              name=f"w2b{b}")
            nc.sync.dma_start(out=wb, in_=w2[b * 128:(b + 1) * 128, :, :])
            w2b.append(wb)

        b2b = singles.tile([128, D], FP)
        nc.sync.dma_start(out=b2b, in_=b2)

        emit_xload(1, nc.sync)
        emit_xload(2, nc.scalar)   # WAR on slot 0 delays this past FC1(c0)
        emit_xload(3, nc.scalar)   # WAR on slot 1 delays this past FC1(c1)

        wtile = singles.tile([128, 128], FP16)
        nc.vector.memset(wtile, 0.0)

        # ---- HAM warm-up: dependency-free real matmuls on a zero tile
        # bring the PE clock gate to 8/8 during the DMA-bound lead-in.
        for j in range(N_JUNK):
            pt = psum.tile([128, 128], FP, tag="psA", name=f"wu{j}")
            nc.tensor.matmul(pt, lhsT=wtile, rhs=wtile, start=True, stop=True)

        yT = [yt_pool.tile([128, 512], FP16, tag=f"yt{hk}", name=f"yT{hk}")
              for hk in range(N_HK)]

        for c4 in range(N_C4):
            # ---- FC1: yT[hk] = relu(w1.T @ x[c4-chunk] + b1) ----
            for b in range(N_BLK):
                pts = [psum.tile([128, 512], FP, tag="psA",
                                 name=f"psfc1_{c4}_{b}_{hi}")
                       for hi in range(N_HI)]
                for hi in range(N_HI):
                    hk = b * N_HI + hi
                    for ki in range(N_KI):
                        nc.tensor.matmul(
                            pts[hi],
                            lhsT=w1b[b][:, ki, hi * 128:(hi + 1) * 128],
                            rhs=xTc[c4][:, ki, :],
                            start=(ki == 0), stop=(ki == N_KI - 1))
                    nc.scalar.activation(
                        out=yT[hk], in_=pts[hi],
                        func=RELU, bias=b1t[:, hk:hk + 1], scale=1.0)

            # ---- FC2: one full-H PSUM pass per (ti, dc) output tile ----
            for ti in range(N_TG):
                gti = c4 * N_TG + ti
                for dc in range(N_DC):
                    pt = psum.tile([128, 512], FP, tag="psB",
                                   name=f"psfc2_{c4}_{ti}_{dc}")
                    for hk in range(N_HK):
                        nc.tensor.matmul(
                            pt,
                            lhsT=yT[hk][:, ti * 128:(ti + 1) * 128],
                            rhs=w2b[hk // N_HI][:, hk % N_HI,
                                                dc * 512:(dc + 1) * 512],
                            start=(hk == 0), stop=(hk == N_HK - 1))
                    st = st_pool.tile([128, 512], FP, tag="st",
                                      name=f"st{gti}_{dc}")
                    nc.vector.tensor_add(
                        st, pt, b2b[:, dc * 512:(dc + 1) * 512])
                    eng = nc.sync if (gti + dc) % 2 == 0 else nc.scalar
                    eng.dma_start(
                        out=out[gti * 128:(gti + 1) * 128,
                                dc * 512:(dc + 1) * 512],
                        in_=st)


def build_module():
    nc = bacc.Bacc("TRN2", target_bir_lowering=False, debug=False)
    xt = nc.dram_tensor("xt", [N_C4 * 128, N_KI, 512], FP16,
                        kind="ExternalInput").ap()
    w1 = nc.dram_tensor("fc1_w", [N_BLK * 128, N_KI, HB], FP16,
                        kind="ExternalInput").ap()
    b1 = nc.dram_tensor("fc1_b", [128, H // 128], FP,
                        kind="ExternalInput").ap()
    w2 = nc.dram_tensor("fc2_w", [N_BLK * 128, N_HI, D], FP16,
                        kind="ExternalInput").ap()
    b2 = nc.dram_tensor("fc2_b", [128, D], FP, kind="ExternalInput").ap()
    out = nc.dram_tensor("out", [T, D], FP, kind="ExternalOutput").ap()
    with tile.TileContext(nc) as tc:
        _emit_kernel(tc, out, xt, w1, b1, w2, b2)
    nc.compile()
    return nc


_CACHED = None


def kernel(x, fc1_w, fc1_b, fc2_w, fc2_b, _trace=False, _trace_cores=None):
    global _CACHED
    if _CACHED is None:
        _CACHED = build_module()
    nc = _CACHED

    # host-side staging: fp16 cast + repack into the kernel's slab layouts
    x16 = np.asarray(x, dtype=np.float32).astype(np.float16)
    w116 = np.asarray(fc1_w, dtype=np.float32).astype(np.float16)
    w216 = np.asarray(fc2_w, dtype=np.float32).astype(np.float16)
    # x [E,T,D] -> xT [E,D,T] -> [E, k, p, c4, t] -> [E, c4, p, k, t]
    xq = np.ascontiguousarray(
        x16.transpose(0, 2, 1).reshape(E, N_KI, 128, N_C4, 512)
           .transpose(0, 3, 2, 1, 4)).reshape(E, N_C4 * 128, N_KI, 512)
    # w1 [E,D,H] -> [E, k, p, b, h] -> [E, b, p, k, h]
    w1q = np.ascontiguousarray(
        w116.reshape(E, N_KI, 128, N_BLK, HB).transpose(0, 3, 2, 1, 4)
    ).reshape(E, N_BLK * 128, N_KI, HB)
    # w2 [E,H,D] -> [E, b, hk, p, d] -> [E, b, p, hk, d]
    w2q = np.ascontiguousarray(
        w216.reshape(E, N_BLK, N_HI, 128, D).transpose(0, 1, 3, 2, 4)
    ).reshape(E, N_BLK * 128, N_HI, D)
    # b1 pre-transposed to [128, 32] ([p, hk] = b1[hk*128+p]); b2
    # pre-broadcast across partitions to [128, D].
    b1q = np.ascontiguousarray(
        np.asarray(fc1_b, dtype=np.float32)
        .reshape(E, H // 128, 128).transpose(0, 2, 1))
    b2q = np.ascontiguousarray(np.broadcast_to(
        np.asarray(fc2_b, dtype=np.float32).reshape(E, 1, D), (E, 128, D)))

    in_maps = [
        {
            "xt": xq[e],
            "fc1_w": w1q[e],
            "fc1_b": b1q[e],
            "fc2_w": w2q[e],
            "fc2_b": b2q[e],
        }
        for e in range(E)
    ]
    kw = {}
    if _trace:
        kw = dict(trace=True,
                  trace_cores=_trace_cores if _trace_cores is not None else [0])
    res = run_bass_kernel_spmd(nc, in_maps, core_ids=list(range(NCORES)), **kw)
    out = np.stack([res.results[e]["out"] for e in range(E)], axis=0)
    if _trace:
        return out, res
    return out
